# Initial kernel scaffold
#
"""AGCRN cell on 8 TRN2 NeuronCores — node-sharded SPMD Bass kernel.

N=2048 nodes sharded 256/core. Two SPMD launches:
  phase 1 (gate, Co=128):  z_r = sigmoid(avwgcn(concat(X,state)));  out zs=z*state, r
  phase 2 (update, Co=64): hc = tanh(avwgcn(concat(X,zs_full)));    out h = r*state+(1-r)*hc
Host only reshapes/shards between phases.

Layout tricks (no PE transposes anywhere):
  - adjacency numerator computed transposed T[m,n] = max(exp(Ef@Eo.T),1)
    (exp(relu(x)) == max(exp(x),1)); row-softmax denominator den[n] via
    ones-matmul; 1/den folded back into T columns via an outer-product
    broadcast, so graph-conv output needs no further normalization.
  - graph conv emitted directly in apply-ready layout: SXT[c, n, b]
    (lhsT = XS m-chunk column-slice per b, rhs = T m-chunk).
  - per-node weights W[n] = sum_d Eo[n,d]*Wp[d] generated on PE in
    [n, (k,i,o)] layout, bounced through DRAM, DMA'd back per node as
    [i, co] stationary tiles for the per-node apply matmuls.
"""

import numpy as np
import ml_dtypes

from concourse import bacc, mybir
from concourse import tile
from concourse.bass_utils import run_bass_kernel_spmd

NCORES = 8
N, B, CIN, HID, D = 2048, 16, 2, 64, 16
CI = CIN + HID          # 66
NO = N // NCORES        # 256 own nodes per core
NT = NO // 128          # 2 node-tiles
MC = N // 128           # 16 m-chunks
BC = B * CI             # 1056

F32 = mybir.dt.float32
F32R = mybir.dt.float32r
BF16 = mybir.dt.bfloat16
AF = mybir.ActivationFunctionType
ALU = mybir.AluOpType


def build_phase(co, act_func, second):
    nc = bacc.Bacc("TRN2", target_bir_lowering=False, debug=False,
                   num_devices=NCORES)
    eoT = nc.dram_tensor("EoT", [D, NO], F32R, kind="ExternalInput")
    efT = nc.dram_tensor("EfT", [D, N], F32R, kind="ExternalInput")
    wf = nc.dram_tensor("Wf", [D, 2 * CI * co], F32R, kind="ExternalInput")
    bp = nc.dram_tensor("bp", [D, co], F32R, kind="ExternalInput")
    xs = nc.dram_tensor("XS", [N, BC], BF16, kind="ExternalInput")
    xt = nc.dram_tensor("XT", [CI, NO, B], BF16, kind="ExternalInput")
    st = nc.dram_tensor("ST", [HID, NO * B], F32, kind="ExternalInput")
    if second:
        rin = nc.dram_tensor("R", [HID, NO * B], F32, kind="ExternalInput")
        hout = nc.dram_tensor("h", [HID, NO * B], F32, kind="ExternalOutput")
    else:
        zsout = nc.dram_tensor("zs", [HID, NO * B], F32, kind="ExternalOutput")
        rout = nc.dram_tensor("r", [HID, NO * B], F32, kind="ExternalOutput")

    nwf = 2 * CI * co                    # flattened (k,i,o) length
    rows_per = 4 if co == 128 else 6     # W rows per gen chunk; 132 % rows == 0
    chunk = rows_per * co                # 512 (gate) / 384 (upd), both >= 256
    ngc = (2 * CI) // rows_per           # 33 / 22 chunks

    with tile.TileContext(nc) as tc:
        with (
            tc.tile_pool(name="psG", bufs=2, space="PSUM") as psG,
            tc.tile_pool(name="psT", bufs=2, space="PSUM") as psT,
            tc.tile_pool(name="psC", bufs=1, space="PSUM") as psC,
            tc.tile_pool(name="psD", bufs=2, space="PSUM") as psD,
            tc.tile_pool(name="sb", bufs=1) as sb,
            tc.tile_pool(name="sbs", bufs=3) as sbs,
            tc.tile_pool(name="wt", bufs=8) as wtp,
            tc.tile_pool(name="dram", bufs=1, space="DRAM") as dram,
        ):
            # ---- small SBUF-resident inputs
            eoT_s = sb.tile([D, NO], F32R, tag="eoT")
            nc.sync.dma_start(out=eoT_s[:], in_=eoT[:])
            efT_s = sb.tile([D, N], F32R, tag="efT")
            nc.sync.dma_start(out=efT_s[:], in_=efT[:])
            bp_s = sb.tile([D, co], F32R, tag="bp")
            nc.sync.dma_start(out=bp_s[:], in_=bp[:])
            xt_s = sb.tile([CI, NO, B], BF16, tag="xt")
            nc.sync.dma_start(out=xt_s[:], in_=xt[:])
            st_s = sb.tile([HID, NO * B], F32, tag="st")
            nc.sync.dma_start(out=st_s[:], in_=st[:])
            if second:
                r_s = sb.tile([HID, NO * B], F32, tag="r")
                nc.sync.dma_start(out=r_s[:], in_=rin[:])
            ones_c = sb.tile([128, 1], BF16, tag="ones_c")
            nc.vector.memset(ones_c[:], 1.0)
            ones_r = sb.tile([1, 128], F32, tag="ones_r")
            nc.vector.memset(ones_r[:], 1.0)

            # ---- biasT [co, NO] = bp.T @ Eo.T
            bps = psG.tile([128, 512], F32, tag="gen")
            nc.tensor.matmul(bps[:co, :NO], bp_s[:], eoT_s[:],
                             start=True, stop=True)
            biasT = sb.tile([co, NO], F32, tag="biasT")
            nc.vector.tensor_copy(biasT[:], bps[:co, :NO])

            # ---- per-node weight gen:  W[n,(k,i,o)] = sum_d Eo[n,d] Wf[d,:]
            wdram = dram.tile([NO, 2 * CI, co], BF16, tag="wdram")
            for t in range(NT):
                for c in range(ngc):
                    lo = c * chunk
                    wfc = sbs.tile([D, chunk], F32R, tag="wfc")
                    nc.sync.dma_start(out=wfc[:], in_=wf[:, lo:lo + chunk])
                    gp = psG.tile([128, 512], F32, tag="gen")
                    nc.tensor.matmul(gp[:, :chunk],
                                     eoT_s[:, t * 128:(t + 1) * 128],
                                     wfc[:], start=True, stop=True)
                    wc = sbs.tile([128, 512], BF16, tag="wc")
                    nc.vector.tensor_copy(wc[:, :chunk], gp[:, :chunk])
                    nc.sync.dma_start(
                        out=wdram[t * 128:(t + 1) * 128,
                                  c * rows_per:(c + 1) * rows_per, :],
                        in_=wc[:, :chunk].rearrange(
                            "p (r o) -> p r o", r=rows_per))

            # ---- T[m,n] = max(exp(Ef@Eo.T), 1)  (transposed layout, bf16)
            tch = []
            for m in range(MC):
                tp = psT.tile([128, NO], F32, tag="tgen")
                nc.tensor.matmul(tp[:], efT_s[:, m * 128:(m + 1) * 128],
                                 eoT_s[:], start=True, stop=True)
                ts = sb.tile([128, NO], BF16, tag=f"T{m}")
                nc.scalar.activation(ts[:], tp[:], AF.Exp)
                nc.vector.tensor_scalar_max(ts[:], ts[:], 1.0)
                tch.append(ts)

            # ---- den[n] = sum_m T[m,n]; fold 1/den into T
            dp = psC.tile([1, NO], F32, tag="den")
            for m in range(MC):
                nc.tensor.matmul(dp[:], ones_c[:], tch[m][:],
                                 start=(m == 0), stop=(m == MC - 1))
            rrow = sb.tile([1, NO], F32, tag="rrow")
            nc.vector.reciprocal(rrow[:], dp[:])
            rbp = psC.tile([128, NO], F32, tag="rb")
            nc.tensor.matmul(rbp[:], ones_r[:], rrow[:], start=True, stop=True)
            for m in range(MC):
                nc.vector.tensor_tensor(tch[m][:], tch[m][:], rbp[:],
                                        op=ALU.mult)

            # ---- resident XS chunks (graph-conv moving operand source)
            xsc = []
            for m in range(MC):
                xc_ = sb.tile([128, BC], BF16, tag=f"XS{m}")
                nc.sync.dma_start(out=xc_[:], in_=xs[m * 128:(m + 1) * 128, :])
                xsc.append(xc_)

            # ---- graph conv, transposed: SXT[c, n, b], accumulate over m
            sxt = sb.tile([CI, NO, B], BF16, tag="sxt")
            for b in range(B):
                cp = psD.tile([CI, NO], F32, tag="conv")
                for m in range(MC):
                    nc.tensor.matmul(cp[:], xsc[m][:, b * CI:(b + 1) * CI],
                                     tch[m][:], start=(m == 0),
                                     stop=(m == MC - 1))
                nc.vector.tensor_copy(sxt[:, :, b], cp[:])

            # ---- per-node apply + fused bias+activation
            zr = sb.tile([co, NO * B], F32, tag="zr")
            for n in range(NO):
                ap = psG.tile([128, 512], F32, tag="gen")
                w0 = wtp.tile([CI, co], BF16, tag="w0")
                nc.sync.dma_start(out=w0[:], in_=wdram[n, 0:CI, :])
                w1 = wtp.tile([CI, co], BF16, tag="w1")
                nc.scalar.dma_start(out=w1[:], in_=wdram[n, CI:2 * CI, :])
                nc.tensor.matmul(ap[:co, :B], w0[:], xt_s[:, n, :],
                                 start=True, stop=False)
                nc.tensor.matmul(ap[:co, :B], w1[:], sxt[:, n, :],
                                 start=False, stop=True)
                nc.scalar.activation(zr[:, n * B:(n + 1) * B], ap[:co, :B],
                                     act_func, bias=biasT[:, n:n + 1])

            # ---- postlude + outputs
            if second:
                tmp = sb.tile([HID, NO * B], F32, tag="tmp")
                nc.vector.tensor_sub(tmp[:], st_s[:], zr[:])
                nc.vector.tensor_mul(tmp[:], tmp[:], r_s[:])
                nc.vector.tensor_add(tmp[:], tmp[:], zr[:])
                nc.sync.dma_start(out=hout[:], in_=tmp[:])
            else:
                zs_s = sb.tile([HID, NO * B], F32, tag="zs")
                nc.vector.tensor_mul(zs_s[:], zr[:HID, :], st_s[:])
                nc.sync.dma_start(out=zsout[:], in_=zs_s[:])
                nc.sync.dma_start(out=rout[:], in_=zr[HID:, :])
    return nc


_CACHE = {}
TRACE = False
LAST_EXEC_NS = None
LAST_PHASE_NS = []
LAST_TRACE = []


def _phases():
    if "p" not in _CACHE:
        nc1 = build_phase(2 * HID, AF.Sigmoid, False)
        nc1.finalize()
        nc2 = build_phase(HID, AF.Tanh, True)
        nc2.finalize()
        _CACHE["p"] = (nc1, nc2)
    return _CACHE["p"]


def kernel(X, state, E, gate_W, gate_b, upd_W, upd_b):
    X = np.asarray(X, np.float32)
    state = np.asarray(state, np.float32)
    E = np.asarray(E, np.float32)
    bf = ml_dtypes.bfloat16
    nc1, nc2 = _phases()
    cores = list(range(NCORES))

    efT = np.ascontiguousarray(E.T)                       # [16, 2048]
    xin = np.concatenate([X, state], -1)                  # [B, N, 66]
    xs1 = np.ascontiguousarray(
        xin.transpose(1, 0, 2).reshape(N, BC)).astype(bf)
    wf1 = np.ascontiguousarray(np.asarray(gate_W, np.float32).reshape(D, -1))
    wf2 = np.ascontiguousarray(np.asarray(upd_W, np.float32).reshape(D, -1))
    bp1 = np.asarray(gate_b, np.float32)
    bp2 = np.asarray(upd_b, np.float32)

    in1, stl = [], []
    for c in cores:
        s = slice(c * NO, (c + 1) * NO)
        eoT = np.ascontiguousarray(E[s].T)                # [16, 256]
        xtc = np.ascontiguousarray(
            xin[:, s].transpose(2, 1, 0)).astype(bf)      # [66, 256, 16]
        stc = np.ascontiguousarray(
            state[:, s].transpose(2, 1, 0).reshape(HID, NO * B))
        stl.append(stc)
        in1.append(dict(EoT=eoT, EfT=efT, Wf=wf1, bp=bp1, XS=xs1,
                        XT=xtc, ST=stc))
    r1 = run_bass_kernel_spmd(nc1, in1, cores, trace=TRACE)
    res1 = r1.results

    # zs_full [m, b, c] from per-core zs [HID, NO*B] == (c, n, b)
    zs_all = np.concatenate(
        [r["zs"].reshape(HID, NO, B).transpose(1, 2, 0) for r in res1], 0)
    xc = np.concatenate([X.transpose(1, 0, 2), zs_all], 2)  # [N, B, 66]
    xs2 = np.ascontiguousarray(xc.reshape(N, BC)).astype(bf)
    in2 = []
    for c in cores:
        s = slice(c * NO, (c + 1) * NO)
        xtc = np.ascontiguousarray(xc[s].transpose(2, 0, 1)).astype(bf)
        in2.append(dict(EoT=in1[c]["EoT"], EfT=efT, Wf=wf2, bp=bp2,
                        XS=xs2, XT=xtc, ST=stl[c], R=res1[c]["r"]))
    r2 = run_bass_kernel_spmd(nc2, in2, cores, trace=TRACE)
    res2 = r2.results
    if TRACE:
        global LAST_EXEC_NS, LAST_PHASE_NS, LAST_TRACE
        LAST_PHASE_NS = [r1.exec_time_ns, r2.exec_time_ns]
        LAST_TRACE = [r1.instructions_and_trace, r2.instructions_and_trace]
        if r1.exec_time_ns and r2.exec_time_ns:
            LAST_EXEC_NS = r1.exec_time_ns + r2.exec_time_ns

    h = np.concatenate(
        [r["h"].reshape(HID, NO, B).transpose(2, 1, 0) for r in res2], 1)
    return np.ascontiguousarray(h, np.float32)



# revision 5
# speedup vs baseline: 1.0088x; 1.0088x over previous
"""AGCRN cell on 8 TRN2 NeuronCores — node-sharded SPMD Bass kernel.

N=2048 nodes sharded 256/core. Two SPMD launches:
  phase 1 (gate, Co=128):  z_r = sigmoid(avwgcn(concat(X,state)));  out zs=z*state, r
  phase 2 (update, Co=64): hc = tanh(avwgcn(concat(X,zs_full)));    out h = r*state+(1-r)*hc
Host only reshapes/shards between phases.

v2 changes vs baseline (798us):
  - bias folded into the generated per-node weights (extra W row + ones row
    in xt), so activations batch 32 nodes at a time (8 ops vs 256).
  - per-node weight readback batched 16 nodes/DMA (32 DMAs vs 512) --
    the baseline was bound by ~600-800ns/dma_start sequencer+HWDGE cost.
  - W-gen wf loads and wdram writes batched by 16-row groups.
  - XS loaded in one DMA; folded supports T computed once in phase 1,
    bounced via DRAM into phase 2.
  - bulk DMA triggers routed through the idle Pool (gpsimd) sequencer.
"""

import numpy as np
import ml_dtypes

from concourse import bacc, mybir
from concourse import tile
from concourse.bass_utils import run_bass_kernel_spmd

NCORES = 8
N, B, CIN, HID, D = 2048, 16, 2, 64, 16
CI = CIN + HID          # 66
ROWS = 2 * CI + 1       # 133 W rows per node: [k0(66) | bias | k1(66)]
NO = N // NCORES        # 256 own nodes per core
MC = N // 128           # 16 m-chunks
BC = B * CI             # 1056
G = 16                  # nodes per W-readback DMA group
NG = NO // G            # 16 groups

F32 = mybir.dt.float32
F32R = mybir.dt.float32r
BF16 = mybir.dt.bfloat16
AF = mybir.ActivationFunctionType
ALU = mybir.AluOpType


def _row_groups(rows_per):
    """16-row write groups, sub-chunked into <=rows_per matmul chunks."""
    out, lo = [], 0
    while lo < ROWS:
        g = min(16, ROWS - lo)
        subs, s = [], lo
        while s < lo + g:
            c = min(rows_per, lo + g - s)
            subs.append((s, c))
            s += c
        out.append((lo, g, subs))
        lo += g
    return out


def build_phase(co, act_func, second):
    nc = bacc.Bacc("TRN2", target_bir_lowering=False, debug=False,
                   num_devices=NCORES)
    eoT = nc.dram_tensor("EoT", [D, NO], F32R, kind="ExternalInput")
    wf = nc.dram_tensor("Wf", [D, ROWS * co], F32R, kind="ExternalInput")
    xs = nc.dram_tensor("XS", [N, BC], BF16, kind="ExternalInput")
    xt = nc.dram_tensor("XT", [CI + 1, NO, B], BF16, kind="ExternalInput")
    st = nc.dram_tensor("ST", [HID, NO * B], F32, kind="ExternalInput")
    if second:
        tin = nc.dram_tensor("T", [128, MC * NO], BF16, kind="ExternalInput")
        rin = nc.dram_tensor("R", [HID, NO * B], F32, kind="ExternalInput")
        hout = nc.dram_tensor("h", [HID, NO * B], F32, kind="ExternalOutput")
    else:
        efT = nc.dram_tensor("EfT", [D, N], F32R, kind="ExternalInput")
        tout = nc.dram_tensor("T", [128, MC * NO], BF16, kind="ExternalOutput")
        zsout = nc.dram_tensor("zs", [HID, NO * B], F32, kind="ExternalOutput")
        rout = nc.dram_tensor("r", [HID, NO * B], F32, kind="ExternalOutput")

    rows_per = 512 // co                 # 4 (gate) / 8 (upd)
    groups = _row_groups(rows_per)
    NT = NO // 128                       # 2 node-tiles for W gen

    with tile.TileContext(nc) as tc:
        with (
            tc.tile_pool(name="psG", bufs=2, space="PSUM") as psG,
            tc.tile_pool(name="psT", bufs=2, space="PSUM") as psT,
            tc.tile_pool(name="psC", bufs=1, space="PSUM") as psC,
            tc.tile_pool(name="psD", bufs=2, space="PSUM") as psD,
            tc.tile_pool(name="sb", bufs=1) as sb,
            tc.tile_pool(name="wfg", bufs=2) as wfgp,
            tc.tile_pool(name="wcb", bufs=3) as wcbp,
            tc.tile_pool(name="wt", bufs=6) as wtp,
            tc.tile_pool(name="dram", bufs=1, space="DRAM") as dram,
        ):
            # ---- small SBUF-resident inputs
            eoT_s = sb.tile([D, NO], F32R, tag="eoT")
            nc.sync.dma_start(out=eoT_s[:], in_=eoT[:])
            xt_s = sb.tile([CI + 1, NO, B], BF16, tag="xt")
            nc.gpsimd.dma_start(out=xt_s[:], in_=xt[:])
            st_s = sb.tile([HID, NO * B], F32, tag="st")
            nc.gpsimd.dma_start(out=st_s[:], in_=st[:])
            if second:
                r_s = sb.tile([HID, NO * B], F32, tag="r")
                nc.gpsimd.dma_start(out=r_s[:], in_=rin[:])

            # ---- resident XS (graph-conv moving operand), one DMA
            xsall = sb.tile([128, MC, BC], BF16, tag="xsall")
            nc.gpsimd.dma_start(
                out=xsall[:], in_=xs.rearrange("(m p) c -> p m c", p=128))

            # ---- per-node weight gen: W[n,(r,o)] = sum_d Eo[n,d] Wf[d,:]
            # wdram[t] layout [128, ROWS, co]; readback transposes per group.
            wdram = [dram.tile([128, ROWS, co], BF16, tag=f"wd{t}",
                               name=f"wd{t}") for t in range(NT)]
            cast_flip = 0
            for t in range(NT):
                for (glo, grows, subs) in groups:
                    wfg = wfgp.tile([D, 16 * co], F32R, tag="wfg")
                    if t == 0:
                        nc.sync.dma_start(
                            out=wfg[:, :grows * co],
                            in_=wf[:, glo * co:(glo + grows) * co])
                    else:
                        # reload per t (cheap) to keep pool rotation simple
                        nc.sync.dma_start(
                            out=wfg[:, :grows * co],
                            in_=wf[:, glo * co:(glo + grows) * co])
                    wcb = wcbp.tile([128, 16 * co], BF16, tag="wcb")
                    for (lo, cnt) in subs:
                        gp = psG.tile([128, 512], F32, tag="gen")
                        nc.tensor.matmul(
                            gp[:, :cnt * co],
                            eoT_s[:, t * 128:(t + 1) * 128],
                            wfg[:, (lo - glo) * co:(lo - glo + cnt) * co],
                            start=True, stop=True)
                        dst = wcb[:, (lo - glo) * co:(lo - glo + cnt) * co]
                        if cast_flip % 2 == 0:
                            nc.vector.tensor_copy(dst, gp[:, :cnt * co])
                        else:
                            nc.scalar.activation(dst, gp[:, :cnt * co],
                                                 AF.Copy)
                        cast_flip += 1
                    nc.gpsimd.dma_start(
                        out=wdram[t][:, glo:glo + grows, :],
                        in_=wcb[:, :grows * co].rearrange(
                            "p (r o) -> p r o", o=co))

            # ---- folded supports T[m,n] = max(exp(Ef@Eo.T),1)/den[n]
            tch = sb.tile([128, MC, NO], BF16, tag="tch")
            if second:
                nc.sync.dma_start(out=tch[:], in_=tin[:].rearrange(
                    "p (m n) -> p m n", m=MC))
            else:
                efT_s = sb.tile([D, N], F32R, tag="efT")
                nc.sync.dma_start(out=efT_s[:], in_=efT[:])
                ones_c = sb.tile([128, 1], BF16, tag="ones_c")
                nc.vector.memset(ones_c[:], 1.0)
                ones_r = sb.tile([1, 128], F32, tag="ones_r")
                nc.vector.memset(ones_r[:], 1.0)
                for m in range(MC):
                    tp = psT.tile([128, NO], F32, tag="tgen")
                    nc.tensor.matmul(tp[:], efT_s[:, m * 128:(m + 1) * 128],
                                     eoT_s[:], start=True, stop=True)
                    nc.scalar.activation(tch[:, m, :], tp[:], AF.Exp)
                    nc.vector.tensor_scalar_max(tch[:, m, :], tch[:, m, :],
                                                1.0)
                dp = psC.tile([1, NO], F32, tag="den")
                for m in range(MC):
                    nc.tensor.matmul(dp[:], ones_c[:], tch[:, m, :],
                                     start=(m == 0), stop=(m == MC - 1))
                rrow = sb.tile([1, NO], F32, tag="rrow")
                nc.vector.reciprocal(rrow[:], dp[:])
                rbp = psC.tile([128, NO], F32, tag="rb")
                nc.tensor.matmul(rbp[:], ones_r[:], rrow[:], start=True,
                                 stop=True)
                for m in range(MC):
                    nc.vector.tensor_tensor(tch[:, m, :], tch[:, m, :],
                                            rbp[:], op=ALU.mult)
                nc.gpsimd.dma_start(
                    out=tout[:], in_=tch[:].rearrange("p m n -> p (m n)"))

            # ---- graph conv, transposed: SXT[c, n, b], accumulate over m
            sxt = sb.tile([CI, NO, B], BF16, tag="sxt")
            for b in range(B):
                cp = psD.tile([CI, NO], F32, tag="conv")
                for m in range(MC):
                    nc.tensor.matmul(cp[:], xsall[:, m, b * CI:(b + 1) * CI],
                                     tch[:, m, :], start=(m == 0),
                                     stop=(m == MC - 1))
                nc.vector.tensor_copy(sxt[:, :, b], cp[:])

            # ---- per-node apply; bias comes via W row 66 x ones row in xt
            zr = sb.tile([co, NO * B], F32, tag="zr")
            ap = None
            for g in range(NG):
                t, loc = divmod(g, NG // NT)
                w0 = wtp.tile([CI + 1, G, co], BF16, tag="w0")
                nc.sync.dma_start(
                    out=w0[:],
                    in_=wdram[t][loc * G:(loc + 1) * G, 0:CI + 1, :]
                    .rearrange("g r o -> r g o"))
                w1 = wtp.tile([CI, G, co], BF16, tag="w1")
                nc.scalar.dma_start(
                    out=w1[:],
                    in_=wdram[t][loc * G:(loc + 1) * G, CI + 1:ROWS, :]
                    .rearrange("g r o -> r g o"))
                if g % 2 == 0:
                    ap = psG.tile([128, 512], F32, tag="gen")
                for j in range(G):
                    n = g * G + j
                    col = (n % 32) * B
                    nc.tensor.matmul(ap[:co, col:col + B], w0[:, j, :],
                                     xt_s[:, n, :], start=True, stop=False)
                    nc.tensor.matmul(ap[:co, col:col + B], w1[:, j, :],
                                     sxt[:, n, :], start=False, stop=True)
                if g % 2 == 1:
                    n0 = (g - 1) * G
                    nc.scalar.activation(zr[:, n0 * B:(n0 + 32) * B],
                                         ap[:co, :], act_func)

            # ---- postlude + outputs
            if second:
                tmp = sb.tile([HID, NO * B], F32, tag="tmp")
                nc.vector.tensor_sub(tmp[:], st_s[:], zr[:])
                nc.vector.tensor_mul(tmp[:], tmp[:], r_s[:])
                nc.vector.tensor_add(tmp[:], tmp[:], zr[:])
                nc.sync.dma_start(out=hout[:], in_=tmp[:])
            else:
                zs_s = sb.tile([HID, NO * B], F32, tag="zs")
                nc.vector.tensor_mul(zs_s[:], zr[:HID, :], st_s[:])
                nc.sync.dma_start(out=zsout[:], in_=zs_s[:])
                nc.sync.dma_start(out=rout[:], in_=zr[HID:, :])
    return nc


_CACHE = {}
TRACE = False
LAST_EXEC_NS = None
LAST_PHASE_NS = []
LAST_TRACE = []


def _phases():
    if "p" not in _CACHE:
        nc1 = build_phase(2 * HID, AF.Sigmoid, False)
        nc1.finalize()
        nc2 = build_phase(HID, AF.Tanh, True)
        nc2.finalize()
        _CACHE["p"] = (nc1, nc2)
    return _CACHE["p"]


def _wf_with_bias(W, b, co):
    # W [D,2,CI,co], b [D,co] -> [D, ROWS*co] rows [k0(66) | bias | k1(66)]
    W = np.asarray(W, np.float32)
    b = np.asarray(b, np.float32)
    return np.ascontiguousarray(np.concatenate(
        [W[:, 0].reshape(D, CI * co), b.reshape(D, co),
         W[:, 1].reshape(D, CI * co)], axis=1))


def kernel(X, state, E, gate_W, gate_b, upd_W, upd_b):
    X = np.asarray(X, np.float32)
    state = np.asarray(state, np.float32)
    E = np.asarray(E, np.float32)
    bf = ml_dtypes.bfloat16
    nc1, nc2 = _phases()
    cores = list(range(NCORES))

    efT = np.ascontiguousarray(E.T)                       # [16, 2048]
    xin = np.concatenate([X, state], -1)                  # [B, N, 66]
    xs1 = np.ascontiguousarray(
        xin.transpose(1, 0, 2).reshape(N, BC)).astype(bf)
    wf1 = _wf_with_bias(gate_W, gate_b, 2 * HID)
    wf2 = _wf_with_bias(upd_W, upd_b, HID)
    ones_nb = np.ones((1, NO, B), np.float32)

    in1, stl = [], []
    for c in cores:
        s = slice(c * NO, (c + 1) * NO)
        eoT = np.ascontiguousarray(E[s].T)                # [16, 256]
        xtc = np.ascontiguousarray(np.concatenate(
            [xin[:, s].transpose(2, 1, 0), ones_nb], 0)).astype(bf)
        stc = np.ascontiguousarray(
            state[:, s].transpose(2, 1, 0).reshape(HID, NO * B))
        stl.append(stc)
        in1.append(dict(EoT=eoT, EfT=efT, Wf=wf1, XS=xs1, XT=xtc, ST=stc))
    r1 = run_bass_kernel_spmd(nc1, in1, cores, trace=TRACE)
    res1 = r1.results

    # zs_full [m, b, c] from per-core zs [HID, NO*B] == (c, n, b)
    zs_all = np.concatenate(
        [r["zs"].reshape(HID, NO, B).transpose(1, 2, 0) for r in res1], 0)
    xc = np.concatenate([X.transpose(1, 0, 2), zs_all], 2)  # [N, B, 66]
    xs2 = np.ascontiguousarray(xc.reshape(N, BC)).astype(bf)
    in2 = []
    for c in cores:
        s = slice(c * NO, (c + 1) * NO)
        xtc = np.ascontiguousarray(np.concatenate(
            [xc[s].transpose(2, 0, 1), ones_nb], 0)).astype(bf)
        in2.append(dict(EoT=in1[c]["EoT"], Wf=wf2, XS=xs2, XT=xtc,
                        ST=stl[c], T=res1[c]["T"], R=res1[c]["r"]))
    r2 = run_bass_kernel_spmd(nc2, in2, cores, trace=TRACE)
    res2 = r2.results
    if TRACE:
        global LAST_EXEC_NS, LAST_PHASE_NS, LAST_TRACE
        LAST_PHASE_NS = [r1.exec_time_ns, r2.exec_time_ns]
        LAST_TRACE = [r1.trace_dir if hasattr(r1, 'trace_dir') else None,
                      r2.trace_dir if hasattr(r2, 'trace_dir') else None]
        if r1.exec_time_ns and r2.exec_time_ns:
            LAST_EXEC_NS = r1.exec_time_ns + r2.exec_time_ns

    h = np.concatenate(
        [r["h"].reshape(HID, NO, B).transpose(2, 1, 0) for r in res2], 1)
    return np.ascontiguousarray(h, np.float32)
